# revision 2
# baseline (speedup 1.0000x reference)
"""MultiHeadedAttention Trainium2 Bass kernel.

Full inputs in, full output out. Sharding: 8 cores = 4 batches x 2 head-pairs
(data-parallel over batch, tensor-parallel over the 4 heads). Per core, all
matmuls in bf16 (fp32 PSUM accumulation):
  Q/K projections for its 2 heads      -> [128, 2048] bf16 (chan-major)
  V projection directly transposed     -> vt [m, (h, d+ones)] bf16
  per head/n-half: scoresT[m,n] = K^T Q, exp on ACT (scale=1/8), x[d+1, n]
  accumulated over m in PSUM with vt stationary (ones row gives softmax
  sums), normalize via DMA-broadcast + fast-approx reciprocal, out
  projection per n-half with both heads accumulated in PSUM.

Bias algebra (lets the device skip bk/bv entirely):
  - K bias shifts scores by a per-query constant -> softmax-invariant, drop.
  - V bias adds exactly bv to the normalized x (softmax weights sum to 1),
    so its output contribution is wm @ bv -> host adds (bm + wm @ bv).
Host pre-casts inputs/weights to bf16 and sums the two per-batch partials.

Schedule: input DMAs issue first (sync queue), weights on the scalar queue,
a short 8-matmul warmup burst releases the HAM clock gate during the DMA
ramp, and a tiny dummy exp pulls the ACT table load into the startup phase.
The softmax exp on the Scalar engine (64 x [128,1024] ACTIVATEs ~= 71us) is
the roofline; everything else hides behind it.
"""

import sys

if "/opt/trn_rl_repo" not in sys.path:
    sys.path.insert(0, "/opt/trn_rl_repo")

import numpy as np
import ml_dtypes

BF = ml_dtypes.bfloat16

B, D, N, H = 4, 256, 2048, 4
DIM = D // H  # 64
NW = 4  # 512-wide n windows
MB = 16  # 128-wide m blocks

_CACHE = {}


def _emit(ctx, tc, io):
    import concourse.bass as bass
    import concourse.mybir as mybir

    nc = tc.nc
    f32 = mybir.dt.float32
    bf16 = mybir.dt.bfloat16
    EXP = mybir.ActivationFunctionType.Exp

    const = ctx.enter_context(tc.tile_pool(name="const", bufs=1))
    xin = ctx.enter_context(tc.tile_pool(name="xin", bufs=4))
    big = ctx.enter_context(tc.tile_pool(name="big", bufs=1))
    xpool = ctx.enter_context(tc.tile_pool(name="xpool", bufs=2))
    pb = ctx.enter_context(tc.tile_pool(name="probs", bufs=3))
    work = ctx.enter_context(tc.tile_pool(name="work", bufs=2))
    outp = ctx.enter_context(tc.tile_pool(name="outp", bufs=3))
    psA = ctx.enter_context(tc.tile_pool(name="psA", bufs=3, space="PSUM"))
    psX = ctx.enter_context(tc.tile_pool(name="psX", bufs=1, space="PSUM"))
    dpool = ctx.enter_context(tc.tile_pool(name="dpool", bufs=2, space="DRAM"))

    # ---- input loads first on the sync HWDGE queue so the big transfers
    # start immediately; weights go on the scalar queue (ACT is idle until
    # the first exp anyway).
    xq_t, xk_t, xv_t = [], [], []
    for w in range(NW):
        for name, lst in (("xq", xq_t), ("xk", xk_t), ("xv", xv_t)):
            t = xin.tile([128, 2, 512], bf16, tag=name, name=f"{name}{w}")
            lst.append(t)

    def load_win(lst, name, w):
        src = io[name].rearrange("(c p) n -> p c n", p=128)
        nc.sync.dma_start(lst[w], src[:, :, w * 512 : (w + 1) * 512])

    load_win(xk_t, "xk", 0)
    load_win(xq_t, "xq", 0)
    load_win(xq_t, "xq", 1)
    load_win(xv_t, "xv", 0)

    wqt_sb = const.tile([128, 2, 128], bf16, tag="wqt")
    nc.scalar.dma_start(wqt_sb, io["wqt"].rearrange("(c p) o -> p c o", p=128))
    wkt_sb = const.tile([128, 2, 128], bf16, tag="wkt")
    nc.scalar.dma_start(wkt_sb, io["wkt"].rearrange("(c p) o -> p c o", p=128))
    wvt_sb = const.tile([128, 2, 128], bf16, tag="wvt")
    nc.scalar.dma_start(wvt_sb, io["wvt"].rearrange("(c p) o -> p c o", p=128))
    bq_sb = const.tile([128, 1], f32, tag="bq")
    nc.scalar.dma_start(bq_sb, io["bq"])
    wmt0_sb = const.tile([64, 256], bf16, tag="wmt0")
    nc.scalar.dma_start(wmt0_sb, io["wmt0"])
    wmt1_sb = const.tile([64, 256], bf16, tag="wmt1")
    nc.scalar.dma_start(wmt1_sb, io["wmt1"])

    load_win(xk_t, "xk", 1)
    load_win(xv_t, "xv", 1)
    load_win(xq_t, "xq", 2)
    load_win(xq_t, "xq", 3)
    load_win(xk_t, "xk", 2)
    load_win(xv_t, "xv", 2)
    load_win(xk_t, "xk", 3)
    load_win(xv_t, "xv", 3)

    # ---- small constants / warmup ----
    wu_a = const.tile([128, 128], bf16, tag="wu_a")
    nc.gpsimd.memset(wu_a, 0.0)
    wu_b = const.tile([128, 512], bf16, tag="wu_b")
    nc.gpsimd.memset(wu_b, 0.0)
    dum = const.tile([1, 16], f32, tag="dum")
    nc.gpsimd.memset(dum, 0.0)
    dum_o = const.tile([1, 16], bf16, tag="dum_o")
    # dummy exp: forces the ~2.7us ACT table load during the DMA ramp
    nc.scalar.activation(dum_o, dum, EXP, scale=0.125)

    # PE warmup: HAM clock gate releases (1.2 -> 2.4 GHz) after ~3.4us of
    # sustained matmul activity; 8 cold 512-wide matmuls cover that window.
    wu_ps = psA.tile([128, 1024], f32, tag="ps", name="wu_ps")
    for i in range(8):
        nc.tensor.matmul(wu_ps[:, 0:512], lhsT=wu_a, rhs=wu_b, start=True, stop=True)

    # ---- projections (Q with bias, K bias dropped) + V^T ----
    q_sb = big.tile([128, 2048], bf16, tag="q")
    k_sb = big.tile([128, 2048], bf16, tag="k")

    def proj_step(xt, wt, dst, w, bias=None):
        ps = psA.tile([128, 1024], f32, tag="ps", name=f"psproj{w}")
        nc.tensor.matmul(ps[:, 0:512], lhsT=wt[:, 0, :], rhs=xt[w][:, 0, :], start=True, stop=False)
        nc.tensor.matmul(ps[:, 0:512], lhsT=wt[:, 1, :], rhs=xt[w][:, 1, :], start=False, stop=True)
        d = dst[:, w * 512 : (w + 1) * 512]
        if bias is None:
            nc.vector.tensor_copy(d, ps[:, 0:512])
        else:
            nc.vector.tensor_scalar_add(d, ps[:, 0:512], bias)

    vt = big.tile([128, MB, 2, 65], bf16, tag="vt")
    nc.gpsimd.memset(vt[:, :, :, 64:65], 1.0)

    def vt_step(mb):
        w, off = divmod(mb, 4)
        ms = slice(off * 128, (off + 1) * 128)
        ps = psA.tile([128, 1024], f32, tag="ps", name=f"psvt{mb}")
        pvt = ps[:, 0:128]
        nc.tensor.matmul(pvt, lhsT=xv_t[w][:, 0, ms], rhs=wvt_sb[:, 0, :], start=True, stop=False)
        nc.tensor.matmul(pvt, lhsT=xv_t[w][:, 1, ms], rhs=wvt_sb[:, 1, :], start=False, stop=True)
        nc.vector.tensor_copy(vt[:, mb, :, 0:64], pvt.rearrange("m (h d) -> m h d", h=2))

    proj_step(xk_t, wkt_sb, k_sb, 0)
    proj_step(xq_t, wqt_sb, q_sb, 0, bq_sb)
    proj_step(xq_t, wqt_sb, q_sb, 1, bq_sb)
    for mb in range(4):
        vt_step(mb)
    proj_step(xk_t, wkt_sb, k_sb, 1)
    proj_step(xq_t, wqt_sb, q_sb, 2, bq_sb)
    proj_step(xk_t, wkt_sb, k_sb, 2)
    proj_step(xq_t, wqt_sb, q_sb, 3, bq_sb)
    proj_step(xk_t, wkt_sb, k_sb, 3)
    for mb in range(4, MB):
        vt_step(mb)

    # ---- attention ----
    # PE-order grouping: without explicit deps the scheduler alternates
    # scores and x-accum matmuls, forcing a LDWEIGHTS before every matmul.
    # Enforce [2 scores of iter g+1][2 x-accums of iter g] alternation.
    from concourse.tile_rust import add_dep_helper

    def _raw(inst):
        return getattr(inst, "ins", inst)

    x_sb = [xpool.tile([64, 2048], bf16, tag="x", name=f"x{h}") for h in range(2)]
    sc_groups = []
    xa_groups = []

    def out_group(nh):
        nbase = nh * 1024
        for oc in range(2):
            ocs = slice(oc * 128, (oc + 1) * 128)
            po = psA.tile([128, 1024], f32, tag="ps", name=f"po{nh}_{oc}")
            for h, wmt in ((0, wmt0_sb), (1, wmt1_sb)):
                for j in range(2):
                    nc.tensor.matmul(
                        po[:, j * 512 : (j + 1) * 512],
                        lhsT=wmt[:, ocs],
                        rhs=x_sb[h][:, nbase + j * 512 : nbase + (j + 1) * 512],
                        start=(h == 0),
                        stop=(h == 1),
                    )
            ot = outp.tile([128, 1024], f32, tag="ot", name="ot")
            nc.vector.tensor_copy(ot, po)
            nc.sync.dma_start(io["out"][ocs, nbase : nbase + 1024], ot)

    for h in range(2):
        qh = q_sb[h * 64 : (h + 1) * 64, :]
        kh = k_sb[h * 64 : (h + 1) * 64, :]
        for nh in range(2):
            nbase = nh * 1024
            px = psX.tile([65, 1024], f32, tag="px", name=f"px{h}_{nh}")
            for mb in range(MB):
                pt = pb.tile([128, 1024], bf16, tag="pt", name="pt")
                sc = psA.tile([128, 1024], f32, tag="ps", name="pssc")
                scg = []
                for s2 in range(2):
                    n0 = nbase + s2 * 512
                    scg.append(nc.tensor.matmul(
                        sc[:, s2 * 512 : (s2 + 1) * 512],
                        lhsT=kh[:, mb * 128 : (mb + 1) * 128],
                        rhs=qh[:, n0 : n0 + 512],
                        start=True,
                        stop=True,
                    ))
                nc.scalar.activation(pt, sc, EXP, scale=0.125)
                sc_groups.append(scg)
                xag = []
                for j in range(2):
                    xag.append(nc.tensor.matmul(
                        px[:, j * 512 : (j + 1) * 512],
                        lhsT=vt[:, mb, h, :],
                        rhs=pt[:, j * 512 : (j + 1) * 512],
                        start=(mb == 0),
                        stop=(mb == MB - 1),
                        skip_group_check=True,
                    ))
                xa_groups.append(xag)

            # normalize: broadcast sums via DRAM bounce, fast-approx
            # reciprocal, one fused multiply into the bf16 x tile.
            s_row = work.tile([1, 1024], f32, tag="s_row", name=f"s_row{h}_{nh}")
            nc.vector.tensor_copy(s_row, px[64:65, :])
            s_dram = dpool.tile([1, 1024], f32, tag="s_dram", name=f"s_dram{h}_{nh}")
            nc.sync.dma_start(s_dram, s_row)
            s_bc = work.tile([64, 1024], f32, tag="s_bc", name=f"s_bc{h}_{nh}")
            s_src = bass.AP(
                tensor=s_dram.tensor,
                offset=s_dram.offset,
                ap=[[0, 64]] + list(s_dram.ap[1:]),
            )
            nc.sync.dma_start(s_bc, s_src)
            r_bc = work.tile([64, 1024], f32, tag="r_bc", name=f"r_bc{h}_{nh}")
            nc.vector.reciprocal_approx_fast(r_bc, s_bc)
            nc.vector.tensor_mul(
                x_sb[h][:, nbase : nbase + 1024], px[0:64, :], r_bc
            )
            if h == 1:
                out_group(nh)

    # PE alternation deps: xa[g] after sc[g+1]; sc[g+2] after xa[g]
    G = len(sc_groups)
    for g in range(G):
        if g + 1 < G:
            for m in xa_groups[g]:
                add_dep_helper(_raw(m), _raw(sc_groups[g + 1][-1]), False,
                               "group x-accums after next scores")
        if g + 2 < G:
            for m in sc_groups[g + 2]:
                add_dep_helper(_raw(m), _raw(xa_groups[g][-1]), False,
                               "group scores after prev x-accums")


def _build_nc():
    key = "nc"
    if key in _CACHE:
        return _CACHE[key]
    from contextlib import ExitStack

    import concourse.mybir as mybir
    import concourse.tile as tile
    from concourse import bacc

    f32 = mybir.dt.float32
    bf16 = mybir.dt.bfloat16
    nc = bacc.Bacc("TRN2", target_bir_lowering=False, debug=False, num_devices=8)
    io = {}
    for name, shape, dt_ in (
        ("xq", [256, 2048], bf16),
        ("xk", [256, 2048], bf16),
        ("xv", [256, 2048], bf16),
        ("wqt", [256, 128], bf16),
        ("wkt", [256, 128], bf16),
        ("wvt", [256, 128], bf16),
        ("bq", [128, 1], f32),
        ("wmt0", [64, 256], bf16),
        ("wmt1", [64, 256], bf16),
    ):
        io[name] = nc.dram_tensor(name, shape, dt_, kind="ExternalInput").ap()
    io["out"] = nc.dram_tensor("out", [256, 2048], f32, kind="ExternalOutput").ap()

    with tile.TileContext(nc) as tc:
        with ExitStack() as ctx:
            _emit(ctx, tc, io)
    nc.compile()
    _CACHE[key] = nc
    return nc


def make_in_maps(query, key, value, wq, bq, wk, bk, wv, bv, wm, bm):
    fb = lambda a: np.ascontiguousarray(np.asarray(a, dtype=np.float32)).astype(BF)
    f = lambda a: np.ascontiguousarray(np.asarray(a), dtype=np.float32)
    query, key, value = f(query), f(key), f(value)
    wq, wk, wv, wm = f(wq), f(wk), f(wv), f(wm)
    bq = f(bq)
    in_maps = []
    for c in range(8):
        b, pair = divmod(c, 2)
        hs = (2 * pair, 2 * pair + 1)
        idx = np.array([d * H + h for h in hs for d in range(DIM)])
        m = {
            "xq": fb(query[b]),
            "xk": fb(key[b]),
            "xv": fb(value[b]),
            "wqt": fb(wq[idx].T),
            "wkt": fb(wk[idx].T),
            "wvt": fb(wv[idx].T),
            "bq": f(bq[idx].reshape(128, 1)),
            "wmt0": fb(wm[:, idx[:64]].T),
            "wmt1": fb(wm[:, idx[64:]].T),
        }
        in_maps.append(m)
    return in_maps


def run(in_maps, trace=False, **kw):
    from concourse import bass_utils

    nc = _build_nc()
    return bass_utils.run_bass_kernel_spmd(
        nc, in_maps, core_ids=list(range(8)), trace=trace, **kw
    )


def gather(results, wm, bv, bm):
    wm = np.asarray(wm, dtype=np.float32)
    bv = np.asarray(bv, dtype=np.float32)
    bm = np.asarray(bm, dtype=np.float32)
    bias = bm + wm @ bv  # device skips bv; its output contribution is wm @ bv
    outs = [np.asarray(r["out"], dtype=np.float32) for r in results]
    return np.stack([outs[2 * b] + outs[2 * b + 1] + bias[:, None] for b in range(B)])


def kernel(query, key, value, wq, bq, wk, bk, wv, bv, wm, bm):
    in_maps = make_in_maps(query, key, value, wq, bq, wk, bk, wv, bv, wm, bm)
    res = run(in_maps)
    return gather(res.results, wm, bv, bm)


# revision 7
# speedup vs baseline: 1.4352x; 1.4352x over previous
"""MultiHeadedAttention Trainium2 Bass kernel.

Full inputs in, full output out. Sharding: 8 cores = 4 batches x 2 head-pairs
(data-parallel over batch, tensor-parallel over the 4 heads). Per core, all
matmuls in bf16 (fp32 PSUM accumulation):
  Q/K projections for its 2 heads      -> [128, 2048] bf16 (chan-major)
  V projection directly transposed     -> vt [m, (h, d+ones)] bf16
  per head/n-half: scoresT[m,n] = K^T Q, exp on ACT (scale=1/8), x[d+1, n]
  accumulated over m in PSUM with vt stationary (ones row gives softmax
  sums), normalize via DMA-broadcast + fast-approx reciprocal, out
  projection per n-half with both heads accumulated in PSUM.

Bias algebra (lets the device skip bk/bv entirely):
  - K bias shifts scores by a per-query constant -> softmax-invariant, drop.
  - V bias adds exactly bv to the normalized x (softmax weights sum to 1),
    so its output contribution is wm @ bv -> host adds (bm + wm @ bv).

Schedule notes:
  - The softmax exp on the Scalar engine (64 x [128,1024] ACTIVATEs ~71us)
    is the roofline; everything else must hide behind it.
  - PE runs 2 score-tiles AHEAD of the exp stream (deps force the static
    order [scores g+2][x-accums g]) so the PE never waits on an exp; a PE
    that waits each iteration can never assemble the ~3.4us of sustained
    activity the HAM clock gate needs to release 2.4 GHz.
  - All weights travel in ONE packed [128,1024] DMA; inputs are 3 big tiles
    loaded in 2 half-tensor DMAs each (DMA *issue* costs ~0.7us of engine
    queue time per dma_start, so fewer/bigger is better for the prologue).
  - px (unnormalized x + sums row) is evacuated to SBUF in one [65,1024]
    copy at block end so the single px PSUM buffer frees quickly.
  - Remaining projections / V^T blocks / the first out-projection group are
    emitted inside the attention loop so the static PE order interleaves
    them into PE slack instead of stalling the queue on input DMAs.
"""

import sys

if "/opt/trn_rl_repo" not in sys.path:
    sys.path.insert(0, "/opt/trn_rl_repo")

import numpy as np
import ml_dtypes

BF = ml_dtypes.bfloat16

B, D, N, H = 4, 256, 2048, 4
DIM = D // H  # 64
NW = 4  # 512-wide n windows
MB = 16  # 128-wide m blocks

_CACHE = {}


def _emit(ctx, tc, io):
    import concourse.bass as bass
    import concourse.mybir as mybir

    nc = tc.nc
    f32 = mybir.dt.float32
    bf16 = mybir.dt.bfloat16
    EXP = mybir.ActivationFunctionType.Exp

    const = ctx.enter_context(tc.tile_pool(name="const", bufs=1))
    xin = ctx.enter_context(tc.tile_pool(name="xin", bufs=1))
    big = ctx.enter_context(tc.tile_pool(name="big", bufs=1))
    xpool = ctx.enter_context(tc.tile_pool(name="xpool", bufs=2))
    pb = ctx.enter_context(tc.tile_pool(name="probs", bufs=3))
    work = ctx.enter_context(tc.tile_pool(name="work", bufs=2))
    outp = ctx.enter_context(tc.tile_pool(name="outp", bufs=3))
    psA = ctx.enter_context(tc.tile_pool(name="psA", bufs=3, space="PSUM"))
    psX = ctx.enter_context(tc.tile_pool(name="psX", bufs=1, space="PSUM"))
    dpool = ctx.enter_context(tc.tile_pool(name="dpool", bufs=2, space="DRAM"))

    # ---- input loads: 3 big tiles, 2 half-tensor DMAs each, n-half A first
    xq_sb = xin.tile([128, 2, 2048], bf16, tag="xq")
    xk_sb = xin.tile([128, 2, 2048], bf16, tag="xk")
    xv_sb = xin.tile([128, 2, 2048], bf16, tag="xv")

    def load_half(t, name, hf):
        src = io[name].rearrange("(c p) n -> p c n", p=128)
        s = slice(hf * 1024, (hf + 1) * 1024)
        nc.sync.dma_start(t[:, :, s], src[:, :, s])

    load_half(xk_sb, "xk", 0)
    load_half(xq_sb, "xq", 0)
    load_half(xv_sb, "xv", 0)
    load_half(xq_sb, "xq", 1)
    load_half(xk_sb, "xk", 1)
    load_half(xv_sb, "xv", 1)

    # ---- weights: one packed [128,1024] bf16 blob + the f32 q bias
    wblob = const.tile([128, 1280], bf16, tag="wblob")
    nc.scalar.dma_start(wblob, io["wblob"])
    bq_sb = const.tile([128, 1], f32, tag="bq")
    nc.scalar.dma_start(bq_sb, io["bq"])
    wqt_v = wblob[:, 0:256].rearrange("p (c o) -> p c o", c=2)
    wkt_v = wblob[:, 256:512].rearrange("p (c o) -> p c o", c=2)
    wvt_v = wblob[:, 512:768].rearrange("p (c o) -> p c o", c=2)
    wmt = (wblob[0:64, 768:1024], wblob[0:64, 1024:1280])

    # ---- small constants ----
    wu_a = const.tile([128, 128], bf16, tag="wu_a")
    nc.gpsimd.memset(wu_a, 0.0)
    wu_b = const.tile([128, 512], bf16, tag="wu_b")
    nc.gpsimd.memset(wu_b, 0.0)
    dum = const.tile([1, 16], f32, tag="dum")
    nc.gpsimd.memset(dum, 0.0)
    dum_o = const.tile([1, 16], bf16, tag="dum_o")
    # dummy exp: forces the ~2.7us ACT table load during the DMA ramp
    nc.scalar.activation(dum_o, dum, EXP, scale=0.125)

    # PE warmup: HAM clock gate releases (1.2 -> 2.4 GHz) after ~3.4us of
    # sustained matmul activity; 10 cold 512-wide matmuls cover that window.
    wu_ps = psA.tile([128, 1024], f32, tag="ps", name="wu_ps")
    for i in range(10):
        nc.tensor.matmul(wu_ps[:, 0:512], lhsT=wu_a, rhs=wu_b, start=True, stop=True)

    # ---- projections (Q with bias, K bias dropped) + V^T ----
    q_sb = big.tile([128, 2048], bf16, tag="q")
    k_sb = big.tile([128, 2048], bf16, tag="k")

    def proj_step(xt, wt, dst, w, bias=None):
        ps = psA.tile([128, 1024], f32, tag="ps", name=f"psproj{w}")
        s = slice(w * 512, (w + 1) * 512)
        nc.tensor.matmul(ps[:, 0:512], lhsT=wt[:, 0, :], rhs=xt[:, 0, s], start=True, stop=False)
        nc.tensor.matmul(ps[:, 0:512], lhsT=wt[:, 1, :], rhs=xt[:, 1, s], start=False, stop=True)
        d = dst[:, s]
        if bias is None:
            nc.vector.tensor_copy(d, ps[:, 0:512])
        else:
            nc.vector.tensor_scalar_add(d, ps[:, 0:512], bias)

    vt = big.tile([128, MB, 2, 65], bf16, tag="vt")
    nc.gpsimd.memset(vt[:, :, :, 64:65], 1.0)

    def vt_step(mb):
        ms = slice(mb * 128, (mb + 1) * 128)
        ps = psA.tile([128, 1024], f32, tag="ps", name=f"psvt{mb}")
        pvt = ps[:, 0:128]
        nc.tensor.matmul(pvt, lhsT=xv_sb[:, 0, ms], rhs=wvt_v[:, 0, :], start=True, stop=False)
        nc.tensor.matmul(pvt, lhsT=xv_sb[:, 1, ms], rhs=wvt_v[:, 1, :], start=False, stop=True)
        nc.vector.tensor_copy(vt[:, mb, :, 0:64], pvt.rearrange("m (h d) -> m h d", h=2))

    proj_step(xk_sb, wkt_v, k_sb, 0)
    proj_step(xq_sb, wqt_v, q_sb, 0, bq_sb)
    proj_step(xq_sb, wqt_v, q_sb, 1, bq_sb)
    for mb in range(4):
        vt_step(mb)

    # ---- attention ----
    from concourse.tile_rust import add_dep_helper

    def _raw(inst):
        return getattr(inst, "ins", inst)

    x_sb = [xpool.tile([64, 2048], bf16, tag="x", name=f"x{h}") for h in range(2)]
    sc_groups = []
    xa_groups = []
    po_state = {}

    def attn_iter(h, nh, mb, px):
        qh = q_sb[h * 64 : (h + 1) * 64, :]
        kh = k_sb[h * 64 : (h + 1) * 64, :]
        nbase = nh * 1024
        pt = pb.tile([128, 1024], bf16, tag="pt", name="pt")
        sc = psA.tile([128, 1024], f32, tag="ps", name="pssc")
        scg = []
        for s2 in range(2):
            n0 = nbase + s2 * 512
            scg.append(nc.tensor.matmul(
                sc[:, s2 * 512 : (s2 + 1) * 512],
                lhsT=kh[:, mb * 128 : (mb + 1) * 128],
                rhs=qh[:, n0 : n0 + 512],
                start=True,
                stop=True,
            ))
        nc.scalar.activation(pt, sc, EXP, scale=0.125)
        sc_groups.append(scg)
        xag = []
        for j in range(2):
            xag.append(nc.tensor.matmul(
                px[:, j * 512 : (j + 1) * 512],
                lhsT=vt[:, mb, h, :],
                rhs=pt[:, j * 512 : (j + 1) * 512],
                start=(mb == 0),
                stop=(mb == MB - 1),
                skip_group_check=True,
            ))
        xa_groups.append(xag)

    def normalize(h, nh, px):
        # evacuate px (64 x rows + sums row) in one copy, then broadcast the
        # sums via a DRAM bounce, fast-approx reciprocal, one fused multiply
        nbase = nh * 1024
        xe = work.tile([65, 1024], f32, tag="xe", name=f"xe{h}_{nh}")
        nc.vector.tensor_copy(xe, px)
        s_dram = dpool.tile([1, 1024], f32, tag="s_dram", name=f"s_dram{h}_{nh}")
        nc.sync.dma_start(s_dram, xe[64:65, :])
        s_bc = work.tile([64, 1024], f32, tag="s_bc", name=f"s_bc{h}_{nh}")
        s_src = bass.AP(
            tensor=s_dram.tensor,
            offset=s_dram.offset,
            ap=[[0, 64]] + list(s_dram.ap[1:]),
        )
        nc.sync.dma_start(s_bc, s_src)
        r_bc = work.tile([64, 1024], f32, tag="r_bc", name=f"r_bc{h}_{nh}")
        nc.vector.reciprocal_approx_fast(r_bc, s_bc)
        nc.vector.tensor_mul(x_sb[h][:, nbase : nbase + 1024], xe[0:64, :], r_bc)

    def po_mms(nh, oc, h):
        key = (nh, oc)
        if key not in po_state:
            po_state[key] = psA.tile([128, 1024], f32, tag="ps", name=f"po{nh}_{oc}")
        po = po_state[key]
        nbase = nh * 1024
        for j in range(2):
            nc.tensor.matmul(
                po[:, j * 512 : (j + 1) * 512],
                lhsT=wmt[h][:, oc * 128 : (oc + 1) * 128],
                rhs=x_sb[h][:, nbase + j * 512 : nbase + (j + 1) * 512],
                start=(h == 0),
                stop=(h == 1),
            )

    def po_out(nh, oc):
        po = po_state.pop((nh, oc))
        ot = outp.tile([128, 1024], f32, tag="ot", name="ot")
        nc.vector.tensor_copy(ot, po)
        nc.sync.dma_start(
            io["out"][oc * 128 : (oc + 1) * 128, nh * 1024 : (nh + 1) * 1024], ot
        )

    # interleaved emission: remaining projections / V^T inside block 0, the
    # nh0 out-projection inside block 3
    fill = {
        (0, 4): lambda: (proj_step(xk_sb, wkt_v, k_sb, 1),
                         vt_step(4), vt_step(5), vt_step(6), vt_step(7)),
        (0, 8): lambda: (proj_step(xq_sb, wqt_v, q_sb, 2, bq_sb),
                         proj_step(xk_sb, wkt_v, k_sb, 2)),
        (0, 12): lambda: (proj_step(xq_sb, wqt_v, q_sb, 3, bq_sb),
                          proj_step(xk_sb, wkt_v, k_sb, 3),
                          vt_step(8), vt_step(9), vt_step(10), vt_step(11)),
        (1, 0): lambda: (vt_step(12), vt_step(13), vt_step(14), vt_step(15)),
    }

    blocks = [(0, 0), (0, 1), (1, 0), (1, 1)]
    for bi, (h, nh) in enumerate(blocks):
        px = psX.tile([65, 1024], f32, tag="px", name=f"px{h}_{nh}")
        for mb in range(MB):
            if (bi, mb) in fill:
                fill[(bi, mb)]()
            attn_iter(h, nh, mb, px)
        normalize(h, nh, px)

    # tail: out-projection
    for nh in range(2):
        for oc in range(2):
            po_mms(nh, oc, 0)
            po_mms(nh, oc, 1)
            po_out(nh, oc)

    # PE run-ahead deps: [scores g+2][x-accums g] alternation so the PE
    # never waits on an exp (and the HAM clock gate stays open).
    G = len(sc_groups)
    for g in range(G):
        if g + 2 < G:
            for m in xa_groups[g]:
                add_dep_helper(_raw(m), _raw(sc_groups[g + 2][-1]), False,
                               "x-accums after scores g+2")
        if g + 3 < G:
            for m in sc_groups[g + 3]:
                add_dep_helper(_raw(m), _raw(xa_groups[g][-1]), False,
                               "scores g+3 after x-accums g")


def _build_nc():
    key = "nc"
    if key in _CACHE:
        return _CACHE[key]
    from contextlib import ExitStack

    import concourse.mybir as mybir
    import concourse.tile as tile
    from concourse import bacc

    f32 = mybir.dt.float32
    bf16 = mybir.dt.bfloat16
    nc = bacc.Bacc("TRN2", target_bir_lowering=False, debug=False, num_devices=8)
    io = {}
    for name, shape, dt_ in (
        ("xq", [256, 2048], bf16),
        ("xk", [256, 2048], bf16),
        ("xv", [256, 2048], bf16),
        ("wblob", [128, 1280], bf16),
        ("bq", [128, 1], f32),
    ):
        io[name] = nc.dram_tensor(name, shape, dt_, kind="ExternalInput").ap()
    io["out"] = nc.dram_tensor("out", [256, 2048], f32, kind="ExternalOutput").ap()

    with tile.TileContext(nc) as tc:
        with ExitStack() as ctx:
            _emit(ctx, tc, io)
    nc.compile()
    _CACHE[key] = nc
    return nc


def make_in_maps(query, key, value, wq, bq, wk, bk, wv, bv, wm, bm):
    fb = lambda a: np.ascontiguousarray(np.asarray(a, dtype=np.float32)).astype(BF)
    f = lambda a: np.ascontiguousarray(np.asarray(a), dtype=np.float32)
    query, key, value = f(query), f(key), f(value)
    wq, wk, wv, wm = f(wq), f(wk), f(wv), f(wm)
    bq = f(bq)

    def chan_pack(w, idx):
        # [256,128] (chan, out) -> [128, 2, 128] -> [128, 256] packed
        wt = w[idx].T.reshape(2, 128, 128).transpose(1, 0, 2).reshape(128, 256)
        return wt

    in_maps = []
    for c in range(8):
        b, pair = divmod(c, 2)
        hs = (2 * pair, 2 * pair + 1)
        idx = np.array([d * H + h for h in hs for d in range(DIM)])
        blob = np.concatenate(
            [
                chan_pack(wq, idx),
                chan_pack(wk, idx),
                chan_pack(wv, idx),
                np.pad(
                    np.concatenate([wm[:, idx[:64]].T, wm[:, idx[64:]].T], axis=1),
                    ((0, 64), (0, 0)),
                ),
            ],
            axis=1,
        )
        m = {
            "xq": fb(query[b]),
            "xk": fb(key[b]),
            "xv": fb(value[b]),
            "wblob": fb(blob),
            "bq": f(bq[idx].reshape(128, 1)),
        }
        in_maps.append(m)
    return in_maps


def run(in_maps, trace=False, **kw):
    from concourse import bass_utils

    nc = _build_nc()
    return bass_utils.run_bass_kernel_spmd(
        nc, in_maps, core_ids=list(range(8)), trace=trace, **kw
    )


def gather(results, wm, bv, bm):
    wm = np.asarray(wm, dtype=np.float32)
    bv = np.asarray(bv, dtype=np.float32)
    bm = np.asarray(bm, dtype=np.float32)
    bias = bm + wm @ bv  # device skips bv; its output contribution is wm @ bv
    outs = [np.asarray(r["out"], dtype=np.float32) for r in results]
    return np.stack([outs[2 * b] + outs[2 * b + 1] + bias[:, None] for b in range(B)])


def kernel(query, key, value, wq, bq, wk, bk, wv, bv, wm, bm):
    in_maps = make_in_maps(query, key, value, wq, bq, wk, bk, wv, bv, wm, bm)
    res = run(in_maps)
    return gather(res.results, wm, bv, bm)
